# revision 14
# baseline (speedup 1.0000x reference)
"""Belief-propagation kernel for 8 Trainium2 NeuronCores (Bass/Tile).

Strategy:
  - Undirected edges are partitioned across the 8 cores with a greedy
    balanced assignment so that every (node, core) pair has at most C1
    incident edges.
  - Each core holds a padded "grid": node n gets C1 slots; slot (n, k)
    stores the message on the k-th local edge incident to n, in BOTH
    directions: D = msg(j->n) (incoming), A = msg(n->j) (outgoing).
    With this dual-array layout every BP update is a regular strided
    operation except one: fetching psi[j] of the other endpoint, which
    is done with the GPSIMD dma_gather instruction from a pair-packed
    psi table in DRAM (int16 indices => two nodes per 256B table row,
    parity-select on chip).
  - Per-node log-prod sums (segment sums) are plain strided reductions
    over the C1 slots; per-iteration partials are AllReduce'd across
    the 8 cores (psum of node partials).
"""

import numpy as np

N_NODES = 50000
NP = 50048            # padded to 128*391
NCOL = 391            # NP // 128
M_UND = 800000
E_DIR = 2 * M_UND
Q = 8
NCORES = 8
P = 128
MAX_ITER = 10


def _greedy_assign(a, b):
    """Assign each undirected edge {a_u, b_u} to one of 8 cores, balancing
    the per-(node, core) incident-edge counts."""
    load = np.zeros((N_NODES, NCORES), dtype=np.int32)
    core = np.empty(len(a), dtype=np.int8)
    la = load  # alias
    a_l = a.tolist()
    b_l = b.tolist()
    for u in range(len(a_l)):
        i = a_l[u]
        j = b_l[u]
        s = la[i] + la[j]
        c = int(np.argmin(s))
        core[u] = c
        la[i, c] += 1
        la[j, c] += 1
    return core, int(load.max())


def _build_core_data(c, core_of, a, b, message_init, C1, SCOL, NCH, CHCOLS):
    """Build one core's grid arrays + gather metadata."""
    import ml_dtypes

    sel = np.nonzero(core_of == c)[0]
    e_fwd = sel                      # directed edge a->b  (dst = b)
    e_bwd = sel + M_UND              # directed edge b->a  (dst = a)
    dst_nodes = np.concatenate([b[sel], a[sel]])
    oth_nodes = np.concatenate([a[sel], b[sel]])
    eids = np.concatenate([e_fwd, e_bwd])          # edge whose dst = dst_nodes
    revids = np.concatenate([e_bwd, e_fwd])        # reverse edge

    # slot position k within each (node) group, in stable order
    order = np.argsort(dst_nodes, kind="stable")
    sn = dst_nodes[order]
    # rank within group
    first = np.searchsorted(sn, sn)  # index of first occurrence? no - use diff
    # compute rank via grouped arange
    counts = np.bincount(sn, minlength=NP)
    assert counts.max() <= C1
    starts = np.concatenate([[0], np.cumsum(counts)[:-1]])
    rank_sorted = np.arange(len(sn)) - starts[sn]
    k_of = np.empty(len(sn), dtype=np.int64)
    k_of[order] = rank_sorted

    n_of = dst_nodes
    pp = n_of % P
    cc = n_of // P
    scol = cc * C1 + k_of

    S8 = SCOL * Q
    D0 = np.zeros((P, SCOL, Q), np.float32)
    A0 = np.zeros((P, SCOL, Q), np.float32)
    mA = np.zeros((P, SCOL), np.float32)
    mB = np.zeros((P, SCOL), np.float32)
    gidx_grid = np.zeros((P, SCOL), np.int64)

    D0[pp, scol] = message_init[eids]
    A0[pp, scol] = message_init[revids]
    par = (oth_nodes % 2).astype(np.float32)
    mA[pp, scol] = 1.0 - par
    mB[pp, scol] = par
    m01 = mA + mB
    gidx_grid[pp, scol] = oth_nodes // 2

    # gather index list per chunk g (CHCOLS slot-cols each):
    # local index kl inside chunk -> slot (partition kl%128, scol g*CHCOLS + kl//128)
    # wrapped int16 tile: position [kl%16, kl//16]
    gidx_chunks = np.zeros((NCH, P, (CHCOLS * P) // 16), np.int16)
    for g in range(NCH):
        ncols = min(CHCOLS, SCOL - g * CHCOLS)
        ni = ncols * P
        kl = np.arange(ni)
        vals = gidx_grid[kl % P, g * CHCOLS + kl // P].astype(np.int16)
        wrapped = np.zeros((16, (CHCOLS * P) // 16), np.int16)
        wrapped[kl % 16, kl // 16] = vals
        gidx_chunks[g] = np.tile(wrapped, (8, 1))

    bf16 = np.float16
    return {
        "D0": D0.reshape(P, S8).astype(bf16),
        "A0": A0.reshape(P, S8).astype(bf16),
        "mA": mA.astype(bf16),
        "m01": m01.astype(bf16),
        "gidx": gidx_chunks,
        # unpack metadata (host side)
        "_eids": eids, "_pp": pp, "_scol": scol,
    }


def _build_nc(C1, SCOL, NCH, CHCOLS, w, beta_over_n):
    import concourse.bass as bass
    import concourse.bacc as bacc
    import concourse.mybir as mybir
    import concourse.tile as tile

    f32 = mybir.dt.float32
    bf16 = mybir.dt.float16
    i16 = mybir.dt.int16
    AF = mybir.ActivationFunctionType
    ALU = mybir.AluOpType
    AX = mybir.AxisListType

    S8 = SCOL * Q
    NPAIR = NP // 2          # 25024 pair-rows in the psi table
    CHN = CHCOLS * P // 16   # int16 idx tile free size per chunk

    nc = bacc.Bacc("TRN2", target_bir_lowering=False, debug=False,
                   num_devices=NCORES, num_swdge_queues=4)

    D_in = nc.dram_tensor("D0", [P, S8], bf16, kind="ExternalInput")
    A_in = nc.dram_tensor("A0", [P, S8], bf16, kind="ExternalInput")
    mA_in = nc.dram_tensor("mA", [P, SCOL], bf16, kind="ExternalInput")
    m01_in = nc.dram_tensor("m01", [P, SCOL], bf16, kind="ExternalInput")
    gidx_in = nc.dram_tensor("gidx", [NCH, P, CHN], i16, kind="ExternalInput")
    deg_in = nc.dram_tensor("deg", [P, NCOL], f32, kind="ExternalInput")
    h0_in = nc.dram_tensor("h0", [P, Q], f32, kind="ExternalInput")

    D_out = nc.dram_tensor("Dout", [P, S8], bf16, kind="ExternalOutput")
    psi_out = nc.dram_tensor("psiout", [P, NCOL * Q], f32, kind="ExternalOutput")

    tblD = nc.dram_tensor("tblD", [2 * NPAIR, 64], f32)

    def bcast(ap, dims):
        """manual AP with explicit [step, count] dims"""
        return bass.AP(ap.tensor, ap.offset, dims)

    with tile.TileContext(nc) as tc:
        with (
            tc.tile_pool(name="state", bufs=1) as state,
            tc.tile_pool(name="lp", bufs=2) as lppool,
            tc.tile_pool(name="g2", bufs=3) as g2pool,
            tc.tile_pool(name="gi", bufs=3) as gipool,
            tc.tile_pool(name="wk", bufs=2) as wk,
            tc.tile_pool(name="zz", bufs=2) as zz,
            tc.tile_pool(name="psum", bufs=2, space="PSUM") as psump,
            tc.tile_pool(name="dram", bufs=2, space="DRAM") as dram,
        ):
            Dg = state.tile([P, SCOL, Q], bf16)
            Ag = state.tile([P, SCOL, Q], bf16)
            mAt = state.tile([P, SCOL], bf16)
            m01t = state.tile([P, SCOL], bf16)
            psi = state.tile([P, NCOL, Q], f32)
            nlp = state.tile([P, NCOL, Q], f32)
            hrow = state.tile([P, Q], f32)
            degt = state.tile([P, NCOL], f32)
            ones128 = state.tile([P, 1], f32)
            eps_t = state.tile([P, 1], f32)

            nc.sync.dma_start(out=Dg[:], in_=D_in.ap().rearrange("p (s q) -> p s q", q=Q))
            nc.sync.dma_start(out=Ag[:], in_=A_in.ap().rearrange("p (s q) -> p s q", q=Q))
            nc.sync.dma_start(out=mAt[:], in_=mA_in[:])
            nc.sync.dma_start(out=m01t[:], in_=m01_in[:])
            nc.sync.dma_start(out=degt[:], in_=deg_in[:])
            nc.sync.dma_start(out=hrow[:], in_=h0_in[:])
            nc.gpsimd.memset(ones128[:], 1.0)
            nc.gpsimd.memset(eps_t[:], 1e-30)

            pp = P * 32  # partition pitch in elements is abstract; use ap dims from tiles instead

            for it in range(MAX_ITER):
                # ---- nlp = segsum_k log(1 + w*D) ----
                LCH = 17  # node-cols per lp chunk
                nch_lp = (NCOL + LCH - 1) // LCH
                for j in range(nch_lp):
                    ncols = min(LCH, NCOL - j * LCH)
                    dch = Dg[:, j * LCH * C1: j * LCH * C1 + ncols * C1, :]
                    lp = lppool.tile([P, LCH * C1, Q], f32, tag="lp")
                    nc.scalar.activation(out=lp[:, :ncols * C1, :], in_=dch,
                                         func=AF.Ln, bias=1.0, scale=w)
                    # reduce over slots: in AP dims ordered (col, q, k) -> X = k
                    lpap = lp[:, :ncols * C1, :]
                    base = lp[:]
                    dims = [base.ap[0], [C1 * Q, ncols], [1, Q], [Q, C1]]
                    nc.vector.tensor_reduce(
                        out=nlp[:, j * LCH: j * LCH + ncols, :],
                        in_=bcast(base, dims),
                        axis=AX.X, op=ALU.add,
                    )
                # ---- AllReduce nlp partials ----
                arin = dram.tile([P, NCOL * Q], f32, tag="arin")
                arout = dram.tile([P, NCOL * Q], f32, tag="arout")
                nc.sync.dma_start(out=arin[:], in_=nlp[:].rearrange("p n q -> p (n q)"))
                nc.gpsimd.collective_compute(
                    "AllReduce", ALU.add,
                    replica_groups=[list(range(NCORES))],
                    ins=[arin.opt()], outs=[arout.opt()],
                )
                nc.sync.dma_start(out=nlp[:].rearrange("p n q -> p (n q)"), in_=arout[:])

                # ---- psi = norm(exp(h + nlp)) ----
                hb = bcast(hrow[:], [hrow[:].ap[0], [0, NCOL], [1, Q]])
                nc.vector.tensor_tensor(out=psi[:], in0=nlp[:], in1=hb, op=ALU.add)
                nc.scalar.activation(out=psi[:], in_=psi[:], func=AF.Exp)
                zn = zz.tile([P, NCOL], f32, tag="zn")
                nc.vector.tensor_reduce(out=zn[:], in_=psi[:], axis=AX.X, op=ALU.add)
                nc.vector.reciprocal(out=zn[:], in_=zn[:])
                znb = bcast(zn[:], [zn[:].ap[0], [1, NCOL], [0, Q]])
                nc.vector.tensor_tensor(out=psi[:], in0=psi[:], in1=znb, op=ALU.mult)

                # ---- h = -(beta/N) * sum_n deg_n * psi_n ----
                wq = zz.tile([P, Q], f32, tag="wq")
                WCH = 98
                wqp = []
                for wc in range((NCOL + WCH - 1) // WCH):
                    wn = min(WCH, NCOL - wc * WCH)
                    wp = wk.tile([P, WCH, Q], f32, tag="wp")
                    dsl = degt[:, wc * WCH: wc * WCH + wn]
                    degb = bcast(dsl, [dsl.ap[0], [1, wn], [0, Q]])
                    nc.vector.tensor_tensor(out=wp[:, :wn, :],
                                            in0=psi[:, wc * WCH: wc * WCH + wn, :],
                                            in1=degb, op=ALU.mult)
                    wqc = zz.tile([P, Q], f32, tag=f"wq{wc}")
                    wpb = wp[:, :wn, :]
                    nc.vector.tensor_reduce(
                        out=wqc[:],
                        in_=bcast(wpb, [wpb.ap[0], [1, Q], [Q, wn]]),
                        axis=AX.X, op=ALU.add,
                    )
                    wqp.append(wqc)
                nc.vector.tensor_tensor(out=wq[:], in0=wqp[0][:], in1=wqp[1][:], op=ALU.add)
                for wc in range(2, len(wqp)):
                    nc.vector.tensor_tensor(out=wq[:], in0=wq[:], in1=wqp[wc][:], op=ALU.add)
                hp1 = psump.tile([1, Q], f32, tag="hp1")
                nc.tensor.matmul(out=hp1[:], lhsT=ones128[:], rhs=wq[:],
                                 start=True, stop=True)
                h1s = zz.tile([1, Q], f32, tag="h1s")
                nc.vector.tensor_copy(out=h1s[:], in_=hp1[:])
                hp2 = psump.tile([P, Q], f32, tag="hp2")
                one_row = zz.tile([1, P], f32, tag="onerow")
                nc.gpsimd.memset(one_row[:], 1.0)
                nc.tensor.matmul(out=hp2[:], lhsT=one_row[:], rhs=h1s[:],
                                 start=True, stop=True)
                nc.scalar.activation(out=hrow[:], in_=hp2[:], func=AF.Identity,
                                     scale=-beta_over_n)

                # ---- pack psi -> pair table in DRAM (even/odd partitions) ----
                tb0 = (it % 2) * NPAIR * 64
                tbl = tblD.ap()
                for par in (0, 1):
                    src = psi[:]
                    sdims = [[src.ap[0][0] * 2, 64], [Q, NCOL], [1, Q]]
                    sap = bass.AP(src.tensor, src.offset + par * src.ap[0][0], sdims)
                    ddims = [[64, 64], [64 * 64, NCOL], [1, Q]]
                    dap = bass.AP(tbl.tensor, tb0 + par * Q, ddims)
                    nc.sync.dma_start(out=dap, in_=sap)

                # ---- gather + message update, chunked ----
                for g in range(NCH):
                    ncols = min(CHCOLS, SCOL - g * CHCOLS)
                    ni = ncols * P
                    c0 = g * CHCOLS
                    gi = gipool.tile([P, CHN], i16, tag="gi")
                    nc.sync.dma_start(out=gi[:, :ni // 16], in_=gidx_in[g, :, :ni // 16])
                    G2 = g2pool.tile([P, CHCOLS, 64], f32, tag="g2")
                    tview = bass.AP(tblD.ap().tensor, tb0, [[64, NPAIR], [1, 64]])
                    nc.gpsimd.dma_gather(
                        G2[:, :ncols, :], tview, gi[:, :ni // 16],
                        ni, ni, 64, single_packet=False, queue_num=g % 4,
                    )
                    # Gf = G2_B + mA*(G2_A - G2_B)
                    mAc = mAt[:, c0:c0 + ncols]
                    mab = [mAc.ap[0], [mAc.ap[1][0], ncols], [0, Q]]
                    Gf = wk.tile([P, CHCOLS, Q], f32, tag="gf")
                    tmp = wk.tile([P, CHCOLS, Q], f32, tag="tmp")
                    nc.vector.tensor_tensor(
                        out=tmp[:, :ncols, :], in0=G2[:, :ncols, 0:Q],
                        in1=G2[:, :ncols, Q:2 * Q], op=ALU.subtract)
                    nc.vector.tensor_tensor(
                        out=tmp[:, :ncols, :], in0=tmp[:, :ncols, :],
                        in1=bcast(mAc, mab), op=ALU.mult)
                    nc.vector.tensor_tensor(
                        out=Gf[:, :ncols, :], in0=tmp[:, :ncols, :],
                        in1=G2[:, :ncols, Q:2 * Q], op=ALU.add)
                    # R_D (before D is overwritten), R_A
                    Dch = Dg[:, c0:c0 + ncols, :]
                    Ach = Ag[:, c0:c0 + ncols, :]
                    RD = wk.tile([P, CHCOLS, Q], f32, tag="rd")
                    RA = wk.tile([P, CHCOLS, Q], f32, tag="ra")
                    nc.scalar.activation(out=RD[:, :ncols, :], in_=Dch,
                                         func=AF.Identity, bias=1.0, scale=w)
                    nc.vector.reciprocal(out=RD[:, :ncols, :], in_=RD[:, :ncols, :])
                    nc.scalar.activation(out=RA[:, :ncols, :], in_=Ach,
                                         func=AF.Identity, bias=1.0, scale=w)
                    nc.vector.reciprocal(out=RA[:, :ncols, :], in_=RA[:, :ncols, :])
                    # D' = norm(Gf * RA)
                    U = Gf
                    nc.vector.tensor_tensor(out=U[:, :ncols, :], in0=Gf[:, :ncols, :],
                                            in1=RA[:, :ncols, :], op=ALU.mult)
                    Zd = zz.tile([P, CHCOLS], f32, tag="zd")
                    nc.vector.tensor_reduce(out=Zd[:, :ncols], in_=U[:, :ncols, :],
                                            axis=AX.X, op=ALU.add)
                    nc.scalar.activation(out=Zd[:, :ncols], in_=Zd[:, :ncols],
                                         func=AF.Identity, bias=eps_t[:])
                    nc.vector.reciprocal(out=Zd[:, :ncols], in_=Zd[:, :ncols])
                    nc.vector.tensor_tensor(out=Zd[:, :ncols], in0=Zd[:, :ncols],
                                            in1=m01t[:, c0:c0 + ncols], op=ALU.mult)
                    zdb = bcast(Zd[:, :ncols],
                                [Zd[:].ap[0], [Zd[:].ap[1][0], ncols], [0, Q]])
                    nc.vector.tensor_tensor(out=Dch, in0=U[:, :ncols, :],
                                            in1=zdb, op=ALU.mult)
                    # A' = norm(psi_own * RD); chunk = ncols/C1 node-cols
                    ncol_nodes = ncols // C1
                    nc0 = c0 // C1
                    pch = psi[:, nc0:nc0 + ncol_nodes, :]
                    pb = bcast(pch, [pch.ap[0], [Q, ncol_nodes], [0, C1], [1, Q]])
                    U2 = RD
                    nc.vector.tensor_tensor(out=U2[:, :ncols, :], in0=pb,
                                            in1=RD[:, :ncols, :], op=ALU.mult)
                    Za = zz.tile([P, CHCOLS], f32, tag="za")
                    nc.vector.tensor_reduce(out=Za[:, :ncols], in_=U2[:, :ncols, :],
                                            axis=AX.X, op=ALU.add)
                    nc.scalar.activation(out=Za[:, :ncols], in_=Za[:, :ncols],
                                         func=AF.Identity, bias=eps_t[:])
                    nc.vector.reciprocal(out=Za[:, :ncols], in_=Za[:, :ncols])
                    zab = bcast(Za[:, :ncols],
                                [Za[:].ap[0], [Za[:].ap[1][0], ncols], [0, Q]])
                    nc.vector.tensor_tensor(out=Ach, in0=U2[:, :ncols, :],
                                            in1=zab, op=ALU.mult)

            nc.sync.dma_start(out=D_out.ap().rearrange("p (s q) -> p s q", q=Q), in_=Dg[:])
            nc.sync.dma_start(out=psi_out.ap().rearrange("p (n q) -> p n q", q=Q), in_=psi[:])

    nc.compile()
    return nc


class kernel_dbg:
    pass


_NC_CACHE = {}


def kernel(edge_index, edge_attr, message_init, psi_init, beta, num_nodes):
    from concourse.bass_utils import run_bass_kernel_spmd

    edge_index = np.asarray(edge_index)
    message_init = np.asarray(message_init, dtype=np.float32)
    psi_init = np.asarray(psi_init, dtype=np.float32)
    beta_f = float(np.asarray(beta))
    n = int(np.asarray(num_nodes))
    assert n == N_NODES
    E = edge_index.shape[1]
    assert E == E_DIR

    src = np.asarray(edge_index[0], dtype=np.int64)
    dst = np.asarray(edge_index[1], dtype=np.int64)
    a = src[:M_UND]
    b = dst[:M_UND]
    # reference guarantees symmetric structure: rev(e) = e + M
    ew_all = np.exp(beta_f * np.asarray(edge_attr, dtype=np.float32)) - 1.0
    w = float(ew_all[0])
    assert np.allclose(ew_all, w, rtol=1e-6), "edge_attr must be uniform"

    deg = np.bincount(dst, minlength=NP).astype(np.float32)

    core_of, C1 = _greedy_assign(a, b)
    C1 = max(C1, 8)
    # chunking
    CH_NODECOLS = 6
    CHCOLS = CH_NODECOLS * C1          # slot-cols per gather chunk
    SCOL = NCOL * C1
    NCH = (SCOL + CHCOLS - 1) // CHCOLS

    per_core = [
        _build_core_data(c, core_of, a, b, message_init, C1, SCOL, NCH, CHCOLS)
        for c in range(NCORES)
    ]

    # h0 = -(beta/N) * sum_n deg_n * psi_init_n
    h0 = -(beta_f / n) * (deg[:N_NODES, None] * psi_init).sum(0)  # [Q]
    h0_tile = np.tile(h0[None, :], (P, 1)).astype(np.float32)
    deg_tile = deg.reshape(NCOL, P).T.copy()  # node n=(c*128+p) -> [p, c]

    key = (C1, SCOL, NCH, CHCOLS, round(w, 9), round(beta_f / n, 12), MAX_ITER)
    if key in _NC_CACHE:
        nc = _NC_CACHE[key]
    else:
        nc = _build_nc(C1, SCOL, NCH, CHCOLS, w, beta_f / n)
        _NC_CACHE[key] = nc

    in_maps = []
    for c in range(NCORES):
        d = per_core[c]
        in_maps.append({
            "D0": np.asarray(d["D0"]),
            "A0": np.asarray(d["A0"]),
            "mA": np.asarray(d["mA"]),
            "m01": np.asarray(d["m01"]),
            "gidx": d["gidx"],
            "deg": deg_tile,
            "h0": h0_tile,
        })

    res = run_bass_kernel_spmd(nc, in_maps, core_ids=list(range(NCORES)),
                               trace=False)

    # ---- unshard ----
    msg = np.zeros((E_DIR, Q), np.float32)
    for c in range(NCORES):
        d = per_core[c]
        Dout = np.asarray(res.results[c]["Dout"]).astype(np.float32)
        Dout = Dout.reshape(P, SCOL, Q)
        msg[d["_eids"]] = Dout[d["_pp"], d["_scol"]]
    kernel_dbg.res = res
    kernel_dbg.per_core = per_core
    kernel_dbg.nc = nc
    kernel_dbg.in_maps = in_maps
    psi_dev = np.asarray(res.results[0]["psiout"]).reshape(P, NCOL, Q)
    psi = psi_dev.transpose(1, 0, 2).reshape(NP, Q)[:N_NODES].copy()
    return msg, psi
